# revision 1
# baseline (speedup 1.0000x reference)
"""Trainium2 Bass kernel for nn_BoundaryLoss (8-core SPMD).

Self-contained: builds the Bass module once, shards full inputs across 8
NeuronCores (data-parallel over batch for the mask/gather stage; anchors x
pos x neg pairwise loss sharded by pos-memory columns), runs via
concourse.bass_utils.run_bass_kernel_spmd, and sums the per-core partial
losses on the host.
"""

import json
import sys
import types
import contextlib
import ctypes

import numpy as np

# ---------------------------------------------------------------------------
# Workaround 1: the walrus compiler in this container accepts only ONE sync
# wait per instruction; Tile's scheduler emits several on join points.  Split
# extra waits into standalone wait-only EventSemaphore instructions inserted
# right before the owning instruction (same engine, same block).
# ---------------------------------------------------------------------------


def _split_multiwaits_json(bir_bytes: bytes) -> bytes:
    j = json.loads(bir_bytes)
    ctr = 0
    changed = False
    for f in j.get("functions", []):
        for bb in f.get("blocks", []):
            new_insts = []
            for inst in bb.get("instructions", []):
                si = inst.get("sync_info")
                ow = (si or {}).get("on_wait") or []
                if len(ow) > 1:
                    changed = True
                    for w in ow[:-1]:
                        ctr += 1
                        new_insts.append(
                            {
                                "debug": inst.get("debug", 0),
                                "engine": inst["engine"],
                                "ins": [],
                                "outs": [],
                                "name": f"I-wsplit-{ctr}",
                                "opcode": "EventSemaphore",
                                "sync_info": {"on_update": [], "on_wait": [w]},
                            }
                        )
                    si["on_wait"] = [ow[-1]]
                new_insts.append(inst)
            bb["instructions"] = new_insts
    if not changed:
        return bir_bytes
    return json.dumps(j).encode()


_patched = False


def _install_patches():
    global _patched
    if _patched:
        return
    from concourse import bass as _bass

    _orig = _bass.Bass.to_json_bytes

    def _to_json_bytes(self):
        return _split_multiwaits_json(_orig(self))

    _bass.Bass.to_json_bytes = _to_json_bytes

    # Workaround 3: EVENT_SEMAPHORE_RANGE_CLEAR encodes a variable-length
    # struct this walrus only accepts for small ranges; clear in chunks.
    from concourse.bass import SemaphoreHandle as _SH, compact_to_ranges as _ctr

    def _clear_and_free(self, sems):
        if not sems:
            return
        sem_nums = [s.num if isinstance(s, _SH) else s for s in sems]
        for sem_range in _ctr(sem_nums):
            assert self._state.free_isdisjoint(sem_range)
            lo = sem_range.start
            while lo < sem_range.stop:
                hi = min(lo + 3, sem_range.stop)
                sub = range(lo, hi)
                self.gpsimd.dma_reset(sub)
                self.gpsimd.sem_clear(sub)
                lo = hi
        self._state.prepend_free_semaphores(sem_nums)
        for poison_set in self._tile_sem_poison_stack:
            poison_set.update(sem_nums)

    _bass.Bass.clear_and_free_semaphores = _clear_and_free

    # Workaround 2: the image lacks antenv.axon_hooks, so trace=True (NTFF
    # profiling) silently degrades.  Provide the module and register the
    # ctypes hook from trn_agent_boot if available.
    try:
        import antenv

        if "antenv.axon_hooks" not in sys.modules:
            m = types.ModuleType("antenv.axon_hooks")
            _store = {}
            m.set_axon_ntff_profile_hook = lambda h: _store.__setitem__("h", h)
            m.get_axon_ntff_profile_hook = lambda: _store.get("h")
            sys.modules["antenv.axon_hooks"] = m
            antenv.axon_hooks = m
            try:
                from trn_agent_boot.trn_boot import _ntff_profile_via_ctypes

                m.set_axon_ntff_profile_hook(
                    _ntff_profile_via_ctypes("/opt/axon/libaxon_pjrt.so")
                )
            except Exception:
                pass
    except Exception:
        pass
    _patched = True


# ---------------------------------------------------------------------------
# Problem constants (hardcoded from the spec)
# ---------------------------------------------------------------------------
B, C, H, W = 8, 21, 512, 512
hh = ww = 128
D = 128
M = 1000
KP = M // 3  # 333
KA = M // 10  # 100
MARGIN = 0.2
NPIX = hh * ww  # 16384 per core
NCORES = 8
PCOLS = M // NCORES  # 125 pos-columns per core

# contribution table layout (rows)
ANC0, ANCW = 0, 256
POS0, POSW = 256, 720
NEG0, NEGW = 976, 720
CTOT = 1696

DVE_COLS = 54  # pairwise columns on DVE; rest on ScalarE

TRACE = False
LAST_EXEC_NS = None

_cache = {}


def _build_module():
    from concourse import bass, tile
    import concourse.mybir as mybir

    dt = mybir.dt
    F32 = dt.float32
    F16 = dt.float16
    Alu = mybir.AluOpType
    Act = mybir.ActivationFunctionType

    nc = bass.Bass(
        trn_type="TRN2", target_bir_lowering=False, debug=False, num_devices=NCORES
    )

    # ---- I/O ----
    preds_t = nc.dram_tensor("preds_t", [128, C * 128], F32, kind="ExternalInput").ap()
    gts_t = nc.dram_tensor("gts_t", [128, 128], dt.int32, kind="ExternalInput").ap()
    embp = nc.dram_tensor("embp", [NPIX, D], F32, kind="ExternalInput").ap()
    posmem = nc.dram_tensor("posmem", [M, D], F32, kind="ExternalInput").ap()
    negmem = nc.dram_tensor("negmem", [M, D], F32, kind="ExternalInput").ap()
    trils_in = nc.dram_tensor("trils", [128, 128], F32, kind="ExternalInput").ap()
    ident_in = nc.dram_tensor("ident", [128, 128], F32, kind="ExternalInput").ap()
    rowiota_in = nc.dram_tensor("rowiota", [128, 1], F32, kind="ExternalInput").ap()
    riota1_in = nc.dram_tensor("riota1", [128, 1], F32, kind="ExternalInput").ap()
    siota3_in = nc.dram_tensor("siota3", [128, 3], F32, kind="ExternalInput").ap()
    prefmask_in = nc.dram_tensor("prefmask", [8, 1], F32, kind="ExternalInput").ap()
    kvec_in = nc.dram_tensor("kvec", [1, 4], F32, kind="ExternalInput").ap()
    poff_in = nc.dram_tensor("poff", [1, 1], dt.int32, kind="ExternalInput").ap()
    out_d = nc.dram_tensor("out", [1, 1], F32, kind="ExternalOutput").ap()

    cnt_loc = nc.dram_tensor("cnt_loc", [1, 4], F32).ap()
    cnt_all = nc.dram_tensor("cnt_all", [8, 4], F32, addr_space="Shared").ap()
    contrib = nc.dram_tensor("contrib", [CTOT, D], F32).ap()
    contrib_o = nc.dram_tensor("contrib_o", [CTOT, D], F32, addr_space="Shared").ap()
    possim_d = nc.dram_tensor("possim_d", [KA, M], F32).ap()

    groups = [list(range(NCORES))]

    with tile.TileContext(nc) as tc:
        with tc.tile_pool(name="cst", bufs=1) as cst, \
             tc.tile_pool(name="wk", bufs=2) as wk, \
             tc.tile_pool(name="big", bufs=1) as big, \
             tc.tile_pool(name="ps", bufs=1, space="PSUM") as ps, \
             tc.tile_pool(name="ps2", bufs=1, space="PSUM") as ps2, \
             tc.tile_pool(name="simp", bufs=1, space="PSUM") as simp:

            # ---------- constant / input loads ----------
            P_sb = big.tile([128, C * 128], F32)
            nc.sync.dma_start(P_sb[:], preds_t)
            G = wk.tile([128, 128], dt.int32)
            nc.sync.dma_start(G[:], gts_t)
            trils = cst.tile([128, 128], F32)
            nc.sync.dma_start(trils[:], trils_in)
            ident = cst.tile([128, 128], F32)
            nc.sync.dma_start(ident[:], ident_in)
            rowiota = cst.tile([128, 1], F32)
            nc.sync.dma_start(rowiota[:], rowiota_in)
            riota1 = cst.tile([128, 1], F32)
            nc.sync.dma_start(riota1[:], riota1_in)
            siota3 = cst.tile([128, 3], F32)
            nc.sync.dma_start(siota3[:], siota3_in)
            prefmask = cst.tile([8, 1], F32)
            nc.sync.dma_start(prefmask[:], prefmask_in)
            kvec = cst.tile([1, 4], F32)
            nc.sync.dma_start(kvec[:], kvec_in)
            poff_sb = cst.tile([1, 1], dt.int32)
            nc.sync.dma_start(poff_sb[:], poff_in)

            zeros = cst.tile([128, 128], F32)
            nc.vector.memset(zeros[:], 0.0)
            ones_t = cst.tile([128, 128], F32)
            nc.vector.memset(ones_t[:], 1.0)
            ones_c = cst.tile([128, 1], F32)
            nc.vector.memset(ones_c[:], 1.0)
            ones_r16 = cst.tile([1, 16], F32)
            nc.vector.memset(ones_r16[:], 1.0)
            ones_r128 = cst.tile([1, 128], F32)
            nc.vector.memset(ones_r128[:], 1.0)

            # memory tables (8 chunks of 128 rows; last holds 104)
            pm = []
            nm = []
            for i in range(8):
                r0 = 128 * i
                rn = min(128, M - r0)
                t1 = wk.tile([128, 128], F32, name=f"pm{i}", tag="pmem", bufs=1)
                nc.sync.dma_start(t1[0:rn, :], posmem[r0 : r0 + rn, :])
                pm.append(t1)
                t2 = wk.tile([128, 128], F32, name=f"nm{i}", tag="nmem", bufs=1)
                nc.sync.dma_start(t2[0:rn, :], negmem[r0 : r0 + rn, :])
                nm.append(t2)

            # zero the contribution table
            for i in range(13):
                nc.sync.dma_start(contrib[128 * i : 128 * (i + 1), :], zeros[:])
            nc.sync.dma_start(contrib[1664:1696, :], zeros[0:32, :])

            # ---------- masks (wrap-dense [128,128], f32 0/1) ----------
            mx = wk.tile([128, 128], F32)
            v = P_sb[:, 128 : C * 128].rearrange("p (c f) -> p f c", c=C - 1)
            nc.vector.tensor_reduce(mx[:], v, axis=mybir.AxisListType.X, op=Alu.max)
            predm = wk.tile([128, 128], F32)
            nc.vector.tensor_tensor(out=predm[:], in0=mx[:], in1=P_sb[:, 0:128], op=Alu.is_gt)
            t1m = wk.tile([128, 128], F32)
            nc.vector.tensor_scalar(out=t1m[:], in0=G[:], scalar1=0.0, scalar2=None, op0=Alu.not_equal)
            t2m = wk.tile([128, 128], F32)
            nc.vector.tensor_scalar(out=t2m[:], in0=G[:], scalar1=255.0, scalar2=None, op0=Alu.not_equal)
            e0m = wk.tile([128, 128], F32)
            nc.vector.tensor_scalar(out=e0m[:], in0=G[:], scalar1=0.0, scalar2=None, op0=Alu.is_equal)
            gtm = wk.tile([128, 128], F32)
            nc.vector.tensor_tensor(out=gtm[:], in0=t1m[:], in1=t2m[:], op=Alu.mult)
            npredm = wk.tile([128, 128], F32)
            nc.vector.tensor_scalar(out=npredm[:], in0=predm[:], scalar1=-1.0, scalar2=1.0, op0=Alu.mult, op1=Alu.add)
            anc_m = wk.tile([128, 128], F32, bufs=1)
            nc.vector.tensor_tensor(out=anc_m[:], in0=predm[:], in1=gtm[:], op=Alu.mult)
            pos_m = wk.tile([128, 128], F32, bufs=1)
            nc.vector.tensor_tensor(out=pos_m[:], in0=gtm[:], in1=npredm[:], op=Alu.mult)
            neg_m = wk.tile([128, 128], F32, bufs=1)
            nc.vector.tensor_tensor(out=neg_m[:], in0=predm[:], in1=e0m[:], op=Alu.mult)
            masks = [anc_m, pos_m, neg_m]

            # ---------- local counts -> AllGather ----------
            rs3 = wk.tile([128, 4], F32)
            nc.vector.memset(rs3[:], 0.0)
            for xi, mk in enumerate(masks):
                nc.vector.tensor_reduce(rs3[:, xi : xi + 1], mk[:], axis=mybir.AxisListType.X, op=Alu.add)
            cnt_ps = ps.tile([1, 4], F32, tag="tiny")
            nc.tensor.matmul(cnt_ps[:], ones_c[:], rs3[:], start=True, stop=True)
            cnt_sb = wk.tile([1, 4], F32)
            nc.scalar.copy(cnt_sb[:], cnt_ps[:])
            nc.sync.dma_start(cnt_loc, cnt_sb[:])
            nc.gpsimd.collective_compute(
                "AllGather", Alu.bypass, replica_groups=groups, ins=[cnt_loc], outs=[cnt_all]
            )
            ca = wk.tile([8, 4], F32)
            nc.sync.dma_start(ca[:], cnt_all)

            # ---------- offsets ----------
            g0_ps = ps.tile([1, 4], F32, tag="tiny")
            nc.tensor.matmul(g0_ps[:], prefmask[:], ca[:], start=True, stop=True)
            g0r = wk.tile([1, 4], F32)
            nc.scalar.copy(g0r[:], g0_ps[:])
            tot_ps = ps.tile([1, 4], F32, tag="tiny")
            nc.tensor.matmul(tot_ps[:], ones_c[0:8, :], ca[:], start=True, stop=True)
            totr = wk.tile([1, 4], F32)
            nc.scalar.copy(totr[:], tot_ps[:])
            cntf = wk.tile([1, 4], F32)  # final counts: min(total, k)
            nc.vector.tensor_tensor(out=cntf[:], in0=totr[:], in1=kvec[:], op=Alu.min)
            srow = wk.tile([1, 4], F32)  # S = clamp(k - g0, 0, 384)
            nc.vector.tensor_tensor(out=srow[:], in0=kvec[:], in1=g0r[:], op=Alu.subtract)
            nc.vector.tensor_scalar(out=srow[:], in0=srow[:], scalar1=0.0, scalar2=384.0, op0=Alu.max, op1=Alu.min)
            g0c = wk.tile([1, 4], F32)  # clamped g0
            nc.vector.tensor_tensor(out=g0c[:], in0=g0r[:], in1=kvec[:], op=Alu.min)
            g0c_i = wk.tile([1, 4], dt.int32)
            nc.vector.tensor_copy(g0c_i[:], g0c[:])

            # broadcast counts to [128,1] columns
            cb_ps = ps.tile([128, 4], F32, tag="tiny")
            nc.tensor.matmul(cb_ps[:], ones_r128[:], cntf[:], start=True, stop=True)
            cntb = wk.tile([128, 4], F32)
            nc.scalar.copy(cntb[:], cb_ps[:])
            sb_ps = ps.tile([128, 4], F32, tag="tiny")
            nc.tensor.matmul(sb_ps[:], ones_r128[:], srow[:], start=True, stop=True)
            s128 = wk.tile([128, 4], F32)
            nc.scalar.copy(s128[:], sb_ps[:])

            # ---------- selection per mask (crossing search + indirect gather) ----------
            specs = [
                (anc_m, 1, ANC0, 0),
                (pos_m, 3, POS0, 1),
                (neg_m, 3, NEG0, 2),
            ]
            for mk, ngrp, base, xi in specs:
                scn = wk.tile([128, 128], F32, name=f"scn{xi}", tag="scn")
                nc.vector.tensor_tensor_scan(scn[:], mk[:], zeros[:], 0.0, Alu.add, Alu.add)
                ro_ps = ps2.tile([128, 1], F32, name=f"rops{xi}", tag="pf")
                nc.tensor.matmul(ro_ps[:], trils[:], scn[:, 127:128], start=True, stop=True)
                rowoff = wk.tile([128, 1], F32, name=f"rowoff{xi}", tag="rowoff")
                nc.scalar.copy(rowoff[:], ro_ps[:])
                Pg = wk.tile([128, 128], F32, name=f"Pg{xi}", tag="Pg")
                nc.vector.tensor_scalar(out=Pg[:], in0=scn[:], scalar1=rowoff[:], scalar2=None, op0=Alu.add)
                roT_ps = ps2.tile([128, 128], F32, name=f"roT{xi}", tag="pf")
                nc.tensor.transpose(roT_ps[0:1, :], rowoff[:], ident[:])
                roT = wk.tile([1, 128], F32, name=f"roTs{xi}", tag="roT")
                nc.scalar.copy(roT[:], roT_ps[0:1, :])
                rb_ps = ps2.tile([128, 128], F32, name=f"rb{xi}", tag="pf")
                nc.tensor.matmul(rb_ps[:], ones_r128[:], roT[:], start=True, stop=True)
                RB = wk.tile([128, 128], F32, name=f"RB{xi}", tag="RB")
                nc.scalar.copy(RB[:], rb_ps[:])
                IDXF = wk.tile([128, 4], F32, name=f"IDXF{xi}", tag="IDXF", bufs=1)
                for cch in range(ngrp):
                    sio = siota3[:, cch : cch + 1]
                    cmp1 = wk.tile([128, 128], F32, name=f"cmp1{xi}{cch}", tag="cmp1")
                    nc.vector.tensor_scalar(out=cmp1[:], in0=RB[:], scalar1=sio, scalar2=None, op0=Alu.is_le)
                    rc = wk.tile([128, 1], F32, name=f"rc{xi}{cch}", tag="rc")
                    nc.vector.tensor_reduce(rc[:], cmp1[:], axis=mybir.AxisListType.X, op=Alu.add)
                    rcT_ps = ps2.tile([128, 128], F32, name=f"rcT{xi}{cch}", tag="pf")
                    nc.tensor.transpose(rcT_ps[0:1, :], rc[:], ident[:])
                    rcT = wk.tile([1, 128], F32, name=f"rcTs{xi}{cch}", tag="rcT")
                    nc.scalar.copy(rcT[:], rcT_ps[0:1, :])
                    rcb_ps = ps2.tile([128, 128], F32, name=f"rcb{xi}{cch}", tag="pf")
                    nc.tensor.matmul(rcb_ps[:], ones_r128[:], rcT[:], start=True, stop=True)
                    Omat = wk.tile([128, 128], F32, name=f"O{xi}{cch}", tag="Omat")
                    nc.vector.tensor_scalar(out=Omat[:], in0=rcb_ps[:], scalar1=riota1[:], scalar2=None, op0=Alu.is_equal)
                    prow_ps = ps2.tile([128, 128], F32, name=f"prw{xi}{cch}", tag="pf")
                    nc.tensor.matmul(prow_ps[:], Omat[:], Pg[:], start=True, stop=True)
                    cmp2 = wk.tile([128, 128], F32, name=f"cmp2{xi}{cch}", tag="cmp2")
                    nc.vector.tensor_scalar(out=cmp2[:], in0=prow_ps[:], scalar1=sio, scalar2=None, op0=Alu.is_le)
                    wc = wk.tile([128, 1], F32, name=f"wc{xi}{cch}", tag="wc")
                    nc.vector.tensor_reduce(wc[:], cmp2[:], axis=mybir.AxisListType.X, op=Alu.add)
                    idxc = wk.tile([128, 1], F32, name=f"idxc{xi}{cch}", tag="idxc")
                    nc.vector.tensor_scalar(out=idxc[:], in0=rc[:], scalar1=128.0, scalar2=-128.0, op0=Alu.mult, op1=Alu.add)
                    nc.vector.tensor_tensor(out=idxc[:], in0=idxc[:], in1=wc[:], op=Alu.add)
                    # invalidate slots >= S (oob index -> skipped by bounds check)
                    vkeep = wk.tile([128, 1], F32, name=f"vk{xi}{cch}", tag="vkeep")
                    nc.vector.tensor_scalar(out=vkeep[:], in0=sio, scalar1=s128[:, xi : xi + 1], scalar2=None, op0=Alu.is_lt)
                    nc.vector.tensor_scalar(out=vkeep[:], in0=vkeep[:], scalar1=-50000.0, scalar2=50000.0, op0=Alu.mult, op1=Alu.add)
                    nc.vector.tensor_tensor(out=idxc[:], in0=idxc[:], in1=vkeep[:], op=Alu.add)
                    nc.vector.tensor_copy(IDXF[:, cch : cch + 1], idxc[:])
                ixT_ps = ps2.tile([4, 128], F32, name=f"ixT{xi}", tag="pf")
                nc.tensor.transpose(ixT_ps[0:ngrp, :], IDXF[:, 0:ngrp], ident[:])
                ixT = wk.tile([4, 128], F32, name=f"ixTs{xi}", tag="ixT", bufs=1)
                nc.scalar.copy(ixT[0:ngrp, :], ixT_ps[0:ngrp, :])
                ixTi = wk.tile([4, 128], dt.int32, name=f"ixTi{xi}", tag="ixTi", bufs=1)
                nc.vector.tensor_copy(ixTi[0:ngrp, :], ixT[0:ngrp, :])
                idxrow = wk.tile([1, 384], dt.int32, name=f"idxrow{xi}", tag="idxrow", bufs=1)
                nc.sync.dma_start(idxrow[0:1, 0 : ngrp * 128], ixTi[0:ngrp, :])
                gat = wk.tile([128, ngrp, 128], F32, name=f"gat{xi}", tag="gat", bufs=1)
                nc.vector.memset(gat[:], 0.0)
                nc.gpsimd.indirect_dma_start(
                    out=gat[:],
                    out_offset=None,
                    in_=embp,
                    in_offset=bass.IndirectOffsetOnAxis(ap=idxrow[0:1, 0 : ngrp * 128], axis=0),
                    bounds_check=NPIX - 1,
                    oob_is_err=False,
                )
                # normalize rows (eps 1e-12)
                for g in range(ngrp):
                    gv = gat[:, g, :]
                    ssq = wk.tile([128, 1], F32, name=f"ssq{xi}{g}", tag="ssq")
                    scr0 = wk.tile([128, 128], F32, name=f"scr0{xi}{g}", tag="scr0")
                    nc.vector.scalar_tensor_tensor(out=scr0[:], in0=gv, scalar=1.0, in1=gv, op0=Alu.mult, op1=Alu.mult, accum_out=ssq[:])
                    nc.scalar.sqrt(ssq[:], ssq[:])
                    nc.vector.tensor_scalar(out=ssq[:], in0=ssq[:], scalar1=1e-12, scalar2=None, op0=Alu.max)
                    nc.vector.reciprocal(ssq[:], ssq[:])
                    nc.vector.tensor_scalar(out=gv, in0=gv, scalar1=ssq[:], scalar2=None, op0=Alu.mult)
                g0reg = nc.values_load(g0c_i[0:1, xi : xi + 1].to_broadcast((1, 1)))
                nc.sync.dma_start(contrib[bass.ds(g0reg + base, ngrp * 128), :], gat[:, 0:ngrp, :])

            # ---------- AllReduce contributions ----------
            nc.gpsimd.collective_compute(
                "AllReduce", Alu.add, replica_groups=groups, ins=[contrib], outs=[contrib_o]
            )

            # ---------- anchors ----------
            canc = wk.tile([128, 128], F32, bufs=1)
            nc.sync.dma_start(canc[0:100, :], contrib_o[0:100, :])
            asq = wk.tile([128, 1], F32)
            ascr = wk.tile([128, 128], F32)
            nc.vector.scalar_tensor_tensor(out=ascr[0:100, :], in0=canc[0:100, :], scalar=1.0, in1=canc[0:100, :], op0=Alu.mult, op1=Alu.mult, accum_out=asq[0:100, :])
            nc.scalar.sqrt(asq[0:100, :], asq[0:100, :])
            nc.vector.tensor_scalar(out=asq[0:100, :], in0=asq[0:100, :], scalar1=1e-8, scalar2=None, op0=Alu.max)
            nc.vector.reciprocal(asq[0:100, :], asq[0:100, :])
            nc.vector.tensor_scalar(out=canc[0:100, :], in0=canc[0:100, :], scalar1=asq[0:100, :], scalar2=None, op0=Alu.mult)
            ancT_ps = simp.tile([128, 100], F32, tag="tsp")
            nc.tensor.transpose(ancT_ps[:], canc[0:100, :], ident[0:100, 0:100])
            ancT = wk.tile([128, 100], F32, bufs=1)
            nc.scalar.copy(ancT[:], ancT_ps[:])

            # ---------- memory tables: merge, normalize (1e-8), transpose ----------
            tabs = []
            for which, mem, cbase in ((0, pm, POS0), (1, nm, NEG0)):
                UT = big.tile([128, M], F32, name=f"UT{which}", tag=f"UT{which}")
                for i in range(8):
                    r0 = 128 * i
                    rn = min(128, M - r0)
                    mt = mem[i]
                    if r0 < KP:
                        newt = wk.tile([128, 128], F32, name=f"nw{which}{i}", tag="newt")
                        nc.sync.dma_start(newt[:], contrib_o[cbase + r0 : cbase + r0 + 128, :])
                        vcol = wk.tile([128, 1], F32, name=f"vc{which}{i}", tag="vcol")
                        nc.vector.tensor_scalar(out=vcol[:], in0=cntb[:, 1 + which : 2 + which], scalar1=float(-r0), scalar2=None, op0=Alu.add)
                        nc.vector.tensor_scalar(out=vcol[:], in0=rowiota[:], scalar1=vcol[:], scalar2=None, op0=Alu.is_lt)
                        vfull = wk.tile([128, 128], dt.uint8, name=f"vf{which}{i}", tag="vfull")
                        nc.vector.tensor_scalar(out=vfull[:], in0=ones_t[:], scalar1=vcol[:], scalar2=None, op0=Alu.mult)
                        nc.vector.copy_predicated(out=mt[:], mask=vfull[:], data=newt[:])
                    msq = wk.tile([128, 1], F32, name=f"msq{which}{i}", tag="msq")
                    mscr = wk.tile([128, 128], F32, name=f"mscr{which}{i}", tag="mscr")
                    nc.vector.scalar_tensor_tensor(out=mscr[0:rn, :], in0=mt[0:rn, :], scalar=1.0, in1=mt[0:rn, :], op0=Alu.mult, op1=Alu.mult, accum_out=msq[0:rn, :])
                    nc.scalar.sqrt(msq[0:rn, :], msq[0:rn, :])
                    nc.vector.tensor_scalar(out=msq[0:rn, :], in0=msq[0:rn, :], scalar1=1e-8, scalar2=None, op0=Alu.max)
                    nc.vector.reciprocal(msq[0:rn, :], msq[0:rn, :])
                    nc.vector.tensor_scalar(out=mt[0:rn, :], in0=mt[0:rn, :], scalar1=msq[0:rn, :], scalar2=None, op0=Alu.mult)
                    tp = simp.tile([128, 128], F32, name=f"tp{which}{i}", tag="tsp")
                    nc.tensor.transpose(tp[0:128, 0:rn], mt[0:rn, :], ident[0:rn, 0:rn])
                    nc.scalar.copy(UT[:, r0 : r0 + rn], tp[0:128, 0:rn])
                tabs.append(UT)
            U_posT, U_negT = tabs

            # ---------- sims ----------
            possim = simp.tile([100, M], F32)
            nc.tensor.matmul(possim[:, 0:512], ancT[:], U_posT[:, 0:512], start=True, stop=True)
            nc.tensor.matmul(possim[:, 512:1000], ancT[:], U_posT[:, 512:1000], start=True, stop=True)
            negsim = simp.tile([100, M], F32)
            nc.tensor.matmul(negsim[:, 0:512], ancT[:], U_negT[:, 0:512], start=True, stop=True)
            nc.tensor.matmul(negsim[:, 512:1000], ancT[:], U_negT[:, 512:1000], start=True, stop=True)
            nbuf = big.tile([100, M], F16)
            nc.scalar.mul(nbuf[:], negsim[:], -1.0)

            possim_sb = big.tile([100, M], F32)
            nc.scalar.copy(possim_sb[:], possim[:])
            nc.sync.dma_start(possim_d, possim_sb[:])
            poffreg = nc.values_load(poff_sb[0:1, 0:1].to_broadcast((1, 1)))
            mypos = wk.tile([100, PCOLS], F32, bufs=1)
            nc.sync.dma_start(mypos[:], possim_d[:, bass.ds(poffreg, PCOLS)])
            validA = wk.tile([128, 1], F32, bufs=1)
            nc.vector.tensor_scalar(out=validA[0:100, :], in0=rowiota[0:100, :], scalar1=cntb[0:100, 0:1], scalar2=None, op0=Alu.is_lt)
            amod = wk.tile([100, PCOLS], F32, bufs=1)
            nc.vector.tensor_scalar(out=amod[:], in0=mypos[:], scalar1=MARGIN + 4.0, scalar2=None, op0=Alu.add)
            nc.vector.tensor_scalar(out=amod[:], in0=amod[:], scalar1=validA[0:100, :], scalar2=4.0, op0=Alu.mult, op1=Alu.subtract)

            # ---------- pairwise relu-sum ----------
            zeros16 = big.tile([100, M], F16)
            nc.vector.memset(zeros16[:], 0.0)
            accD = wk.tile([100, 128], F32, bufs=1)
            nc.vector.memset(accD[:], 0.0)
            accA = wk.tile([100, 128], F32, bufs=1)
            nc.vector.memset(accA[:], 0.0)
            scrD = big.tile([100, M], F16)
            scrA = big.tile([100, M], F16)
            for i in range(PCOLS):
                if i < DVE_COLS:
                    nc.vector.scalar_tensor_tensor(
                        out=scrD[:], in0=nbuf[:], scalar=amod[:, i : i + 1], in1=zeros16[:],
                        op0=Alu.add, op1=Alu.max, accum_out=accD[:, i : i + 1],
                    )
                else:
                    nc.scalar.activation(
                        scrA[:], negsim[:], Act.Relu, bias=amod[:, i : i + 1], scale=-1.0,
                        accum_out=accA[:, i : i + 1],
                    )

            r1 = wk.tile([100, 2], F32, bufs=1)
            nc.vector.tensor_reduce(r1[:, 0:1], accD[:, 0:PCOLS], axis=mybir.AxisListType.X, op=Alu.add)
            nc.vector.tensor_reduce(r1[:, 1:2], accA[:, 0:PCOLS], axis=mybir.AxisListType.X, op=Alu.add)
            rsum = wk.tile([100, 1], F32, bufs=1)
            nc.vector.tensor_tensor(out=rsum[:], in0=r1[:, 0:1], in1=r1[:, 1:2], op=Alu.add)
            tot2 = ps.tile([1, 1], F32, tag="tiny")
            nc.tensor.matmul(tot2[:], rsum[:], ones_c[0:100, :], start=True, stop=True)
            tots = wk.tile([1, 1], F32, bufs=1)
            nc.scalar.copy(tots[:], tot2[:])
            den = wk.tile([1, 1], F32, bufs=1)
            nc.vector.tensor_scalar(out=den[:], in0=cntf[:, 0:1], scalar1=1.0, scalar2=1e6, op0=Alu.max, op1=Alu.mult)
            nc.vector.reciprocal(den[:], den[:])
            nc.vector.tensor_tensor(out=den[:], in0=den[:], in1=tots[:], op=Alu.mult)
            nc.sync.dma_start(out_d, den[:])

    return nc


def _host_shards(preds, embeddings, fsss_gts, pos_memory, neg_memory):
    """Build the 8 per-core input maps."""
    trils = np.tril(np.ones((128, 128), np.float32), -1).T  # lhsT[k,m]=1 iff k<m
    ident = np.eye(128, dtype=np.float32)
    rowiota = np.arange(128, dtype=np.float32).reshape(128, 1)
    riota1 = rowiota + 1.0
    siota3 = np.stack([np.arange(128, dtype=np.float32) + 128 * c for c in range(3)], axis=1)
    kvec = np.array([[KA, KP, KP, 0]], np.float32)

    in_maps = []
    for c in range(NCORES):
        psub = preds[c, :, ::4, ::4]  # [21,128,128]
        preds_t = np.ascontiguousarray(
            psub.transpose(1, 0, 2).reshape(128, C * 128)
        )
        gts_t = np.ascontiguousarray(fsss_gts[c, ::4, ::4]).astype(np.int32)
        embp = np.ascontiguousarray(
            embeddings[c].transpose(1, 2, 0).reshape(NPIX, D)
        )
        prefmask = np.zeros((8, 1), np.float32)
        prefmask[:c] = 1.0
        in_maps.append(
            {
                "preds_t": preds_t.astype(np.float32),
                "gts_t": gts_t,
                "embp": embp.astype(np.float32),
                "posmem": np.ascontiguousarray(pos_memory, dtype=np.float32),
                "negmem": np.ascontiguousarray(neg_memory, dtype=np.float32),
                "trils": trils.astype(np.float32),
                "ident": ident,
                "rowiota": rowiota,
                "riota1": riota1.astype(np.float32),
                "siota3": np.ascontiguousarray(siota3),
                "prefmask": prefmask,
                "kvec": kvec,
                "poff": np.array([[PCOLS * c]], np.int32),
            }
        )
    return in_maps


def kernel(preds, embeddings, fsss_gts, pos_memory, neg_memory):
    global LAST_EXEC_NS
    _install_patches()
    from concourse.bass_utils import run_bass_kernel_spmd

    if "nc" not in _cache:
        _cache["nc"] = _build_module()
    nc = _cache["nc"]

    in_maps = _host_shards(
        np.asarray(preds), np.asarray(embeddings), np.asarray(fsss_gts),
        np.asarray(pos_memory), np.asarray(neg_memory),
    )
    res = run_bass_kernel_spmd(nc, in_maps, list(range(NCORES)), trace=TRACE)
    LAST_EXEC_NS = res.exec_time_ns
    total = np.float32(0.0)
    for r in res.results:
        total = total + r["out"][0, 0]
    return np.float32(total)



# revision 22
# speedup vs baseline: 1.5449x; 1.5449x over previous
"""Trainium2 Bass kernel for nn_BoundaryLoss (8-core SPMD), v2.

Structure (per core):
  t=0   : dummy AllGather (absorbs first-collective rendezvous), input DMAs,
          masks -> counts -> counts AllGather trigger ASAP.
  overlap: selection crossing-search + indirect gathers + row normalize
          (Newton-refined rsqrt: HW ACT sqrt is ~4e-3 sloppy), memory-table
          normalize + transpose to bf16 U tables.  All independent of the
          counts AllGather.
  post-AG: global offsets, indirect-scatter selected rows into a dense
          766-row contribution table, AllReduce it.
  post-AR: merge gathered rows into U tables in transposed space
          (copy_predicated by column-validity), anchors used as-is (already
          unit rows), bf16 sims matmuls, then the pairwise relu-sum with
          f16 tensor_scalar columns on DVE (4x perf mode) split with
          activation columns on ScalarE.
Host sums the 8 per-core partial losses.
"""

import json
import sys
import types

import numpy as np
import ml_dtypes

# ---------------------------------------------------------------------------
# Workaround 1: the walrus compiler in this container accepts only ONE sync
# wait per instruction; Tile's scheduler emits several on join points.  Split
# extra waits into standalone wait-only EventSemaphore instructions inserted
# right before the owning instruction (same engine, same block).
# ---------------------------------------------------------------------------


def _split_multiwaits_json(bir_bytes: bytes) -> bytes:
    j = json.loads(bir_bytes)
    ctr = 0
    changed = False
    for f in j.get("functions", []):
        for bb in f.get("blocks", []):
            new_insts = []
            for inst in bb.get("instructions", []):
                si = inst.get("sync_info")
                ow = (si or {}).get("on_wait") or []
                if len(ow) > 1:
                    changed = True
                    for w in ow[:-1]:
                        ctr += 1
                        new_insts.append(
                            {
                                "debug": inst.get("debug", 0),
                                "engine": inst["engine"],
                                "ins": [],
                                "outs": [],
                                "name": f"I-wsplit-{ctr}",
                                "opcode": "EventSemaphore",
                                "sync_info": {"on_update": [], "on_wait": [w]},
                            }
                        )
                    si["on_wait"] = [ow[-1]]
                new_insts.append(inst)
            bb["instructions"] = new_insts
    if not changed:
        return bir_bytes
    return json.dumps(j).encode()


_patched = False


def _install_patches():
    global _patched
    if _patched:
        return
    from concourse import bass as _bass

    _orig = _bass.Bass.to_json_bytes

    def _to_json_bytes(self):
        return _split_multiwaits_json(_orig(self))

    _bass.Bass.to_json_bytes = _to_json_bytes

    # Workaround 3: EVENT_SEMAPHORE_RANGE_CLEAR encodes a variable-length
    # struct this walrus only accepts for small ranges; clear in chunks.
    from concourse.bass import SemaphoreHandle as _SH, compact_to_ranges as _ctr

    def _clear_and_free(self, sems):
        if not sems:
            return
        sem_nums = [s.num if isinstance(s, _SH) else s for s in sems]
        for sem_range in _ctr(sem_nums):
            assert self._state.free_isdisjoint(sem_range)
            lo = sem_range.start
            while lo < sem_range.stop:
                hi = min(lo + 3, sem_range.stop)
                sub = range(lo, hi)
                self.gpsimd.dma_reset(sub)
                self.gpsimd.sem_clear(sub)
                lo = hi
        self._state.prepend_free_semaphores(sem_nums)
        for poison_set in self._tile_sem_poison_stack:
            poison_set.update(sem_nums)

    _bass.Bass.clear_and_free_semaphores = _clear_and_free

    # Workaround 2: the image lacks antenv.axon_hooks, so trace=True (NTFF
    # profiling) silently degrades.  Provide the module and register the
    # ctypes hook from trn_agent_boot if available.
    try:
        import antenv

        if "antenv.axon_hooks" not in sys.modules:
            m = types.ModuleType("antenv.axon_hooks")
            _store = {}
            m.set_axon_ntff_profile_hook = lambda h: _store.__setitem__("h", h)
            m.get_axon_ntff_profile_hook = lambda: _store.get("h")
            sys.modules["antenv.axon_hooks"] = m
            antenv.axon_hooks = m
            try:
                from trn_agent_boot.trn_boot import _ntff_profile_via_ctypes

                m.set_axon_ntff_profile_hook(
                    _ntff_profile_via_ctypes("/opt/axon/libaxon_pjrt.so")
                )
            except Exception:
                pass
    except Exception:
        pass
    _patched = True


# ---------------------------------------------------------------------------
# Problem constants (hardcoded from the spec)
# ---------------------------------------------------------------------------
B, C, H, W = 8, 21, 512, 512
hh = ww = 128
D = 128
M = 1000
KP = M // 3  # 333
KA = M // 10  # 100
MARGIN = 0.2
NPIX = hh * ww  # 16384 per core
NCORES = 8
PCOLS = M // NCORES  # 125 pos-columns per core

# dense contribution table layout (rows)
ANC0, POS0, NEG0 = 0, KA, KA + KP  # 0, 100, 433
CTOT = KA + 2 * KP  # 766

N_A = 69  # pairwise cols: DVE relu (4x) + PE matmul-accumulate
N_C = 16  # pairwise cols: DVE relu + DVE tensor_tensor add
# remaining PCOLS - N_A - N_C cols: ScalarE activation w/ accum

TRACE = False
LAST_EXEC_NS = None

_cache = {}


def _build_module():
    from concourse import bass, tile
    import concourse.mybir as mybir

    dt = mybir.dt
    F32 = dt.float32
    F16 = dt.float16
    BF16 = dt.bfloat16
    Alu = mybir.AluOpType
    Act = mybir.ActivationFunctionType

    nc = bass.Bass(
        trn_type="TRN2", target_bir_lowering=False, debug=False, num_devices=NCORES
    )

    # ---- I/O ----
    preds_t = nc.dram_tensor("preds_t", [128, C * 128], F32, kind="ExternalInput").ap()
    gts_t = nc.dram_tensor("gts_t", [128, 128], dt.int32, kind="ExternalInput").ap()
    embp = nc.dram_tensor("embp", [NPIX, D], F32, kind="ExternalInput").ap()
    posmem = nc.dram_tensor("posmem", [1024, D], BF16, kind="ExternalInput").ap()
    negmem = nc.dram_tensor("negmem", [1024, D], BF16, kind="ExternalInput").ap()
    trils_in = nc.dram_tensor("trils", [128, 128], F32, kind="ExternalInput").ap()
    ident_in = nc.dram_tensor("ident", [128, 128], F32, kind="ExternalInput").ap()
    identb_in = nc.dram_tensor("identb", [128, 128], BF16, kind="ExternalInput").ap()
    rowiota_in = nc.dram_tensor("rowiota", [128, 1], F32, kind="ExternalInput").ap()
    riota1_in = nc.dram_tensor("riota1", [128, 1], F32, kind="ExternalInput").ap()
    siota3_in = nc.dram_tensor("siota3", [128, 3], F32, kind="ExternalInput").ap()
    iotaf_in = nc.dram_tensor("iotaf", [128, 384], F32, kind="ExternalInput").ap()
    prefmask_in = nc.dram_tensor("prefmask", [8, 1], F32, kind="ExternalInput").ap()
    kvec_in = nc.dram_tensor("kvec", [1, 4], F32, kind="ExternalInput").ap()
    poff_in = nc.dram_tensor("poff", [1, 1], dt.int32, kind="ExternalInput").ap()
    out_d = nc.dram_tensor("out", [1, 1], F32, kind="ExternalOutput").ap()

    cnt_loc = nc.dram_tensor("cnt_loc", [1, 4], F32).ap()
    cnt_all = nc.dram_tensor("cnt_all", [8, 4], F32, addr_space="Shared").ap()
    contrib = nc.dram_tensor("contrib", [CTOT, D], F32).ap()
    contrib_o = nc.dram_tensor("contrib_o", [CTOT, D], F32, addr_space="Shared").ap()
    possim_d = nc.dram_tensor("possim_d", [KA, M], F32).ap()

    groups = [list(range(NCORES))]

    with tile.TileContext(nc) as tc:
        with tc.tile_pool(name="cst", bufs=1) as cst, \
             tc.tile_pool(name="big", bufs=1) as big, \
             tc.tile_pool(name="wk", bufs=2) as wk, \
             tc.tile_pool(name="psA", bufs=1, space="PSUM") as psA, \
             tc.tile_pool(name="psB", bufs=1, space="PSUM") as psB, \
             tc.tile_pool(name="psC", bufs=1, space="PSUM") as psC:

            # ================= SECTION 1: t=0 =================
            G = wk.tile([128, 128], dt.int32, bufs=1)
            nc.sync.dma_start(G[:], gts_t)
            P_sb = big.tile([128, C * 128], F32)
            nc.sync.dma_start(P_sb[:], preds_t)

            trils = cst.tile([128, 128], F32)
            nc.sync.dma_start(trils[:], trils_in)
            ident = cst.tile([128, 128], F32)
            nc.sync.dma_start(ident[:], ident_in)
            identb = cst.tile([128, 128], BF16)
            nc.sync.dma_start(identb[:], identb_in)
            rowiota = cst.tile([128, 1], F32)
            nc.sync.dma_start(rowiota[:], rowiota_in)
            riota1 = cst.tile([128, 1], F32)
            nc.sync.dma_start(riota1[:], riota1_in)
            siota3 = cst.tile([128, 3], F32)
            nc.sync.dma_start(siota3[:], siota3_in)
            iotaf = cst.tile([128, 384], F32)
            nc.sync.dma_start(iotaf[:], iotaf_in)
            prefmask = cst.tile([8, 1], F32)
            nc.sync.dma_start(prefmask[:], prefmask_in)
            kvec = cst.tile([1, 4], F32)
            nc.sync.dma_start(kvec[:], kvec_in)
            poff_sb = cst.tile([1, 1], dt.int32)
            nc.sync.dma_start(poff_sb[:], poff_in)

            # memory tables as single DMAs: [128, 8, 128] bf16
            pmall = big.tile([128, 8, 128], BF16)
            nc.sync.dma_start(
                pmall[:], posmem.rearrange("(i p) d -> p i d", p=128)
            )
            nmall = big.tile([128, 8, 128], BF16)
            nc.sync.dma_start(
                nmall[:], negmem.rearrange("(i p) d -> p i d", p=128)
            )

            zeros = cst.tile([128, 128], F32)
            nc.vector.memset(zeros[:], 0.0)
            ones_c = cst.tile([128, 1], F32)
            nc.vector.memset(ones_c[:], 1.0)
            ones_r128 = cst.tile([1, 128], F32)
            nc.vector.memset(ones_r128[:], 1.0)

            # zero the contribution table (sums with peers via AllReduce)
            for i in range(6):
                r0 = 128 * i
                rn = min(128, CTOT - r0)
                nc.sync.dma_start(contrib[r0 : r0 + rn, :], zeros[0:rn, :])

            # ---------- masks ----------
            # max over channels 1..20 via a tensor_tensor max tree
            t8 = wk.tile([128, 8 * 128], F32, bufs=1)
            nc.vector.tensor_tensor(
                out=t8[:], in0=P_sb[:, 1 * 128 : 9 * 128],
                in1=P_sb[:, 9 * 128 : 17 * 128], op=Alu.max,
            )
            t4 = wk.tile([128, 4 * 128], F32, bufs=1)
            nc.vector.tensor_tensor(
                out=t4[:], in0=t8[:, 0:512], in1=t8[:, 512:1024], op=Alu.max
            )
            r2 = wk.tile([128, 2 * 128], F32, bufs=1)
            nc.vector.tensor_tensor(
                out=r2[:], in0=P_sb[:, 17 * 128 : 19 * 128],
                in1=P_sb[:, 19 * 128 : 21 * 128], op=Alu.max,
            )
            t2 = wk.tile([128, 2 * 128], F32, bufs=1)
            nc.vector.tensor_tensor(
                out=t2[:], in0=t4[:, 0:256], in1=t4[:, 256:512], op=Alu.max
            )
            mx = wk.tile([128, 128], F32, bufs=1)
            nc.vector.tensor_tensor(
                out=mx[:], in0=t2[:, 0:128], in1=t2[:, 128:256], op=Alu.max
            )
            nc.vector.tensor_tensor(
                out=mx[:], in0=mx[:], in1=r2[:, 0:128], op=Alu.max
            )
            nc.vector.tensor_tensor(
                out=mx[:], in0=mx[:], in1=r2[:, 128:256], op=Alu.max
            )
            predm = wk.tile([128, 128], F32, bufs=1)
            nc.vector.tensor_tensor(out=predm[:], in0=mx[:], in1=P_sb[:, 0:128], op=Alu.is_gt)
            t1m = wk.tile([128, 128], F32)
            nc.vector.tensor_scalar(out=t1m[:], in0=G[:], scalar1=0.0, scalar2=None, op0=Alu.not_equal)
            t2m = wk.tile([128, 128], F32)
            nc.vector.tensor_scalar(out=t2m[:], in0=G[:], scalar1=255.0, scalar2=None, op0=Alu.not_equal)
            e0m = wk.tile([128, 128], F32)
            nc.vector.tensor_scalar(out=e0m[:], in0=G[:], scalar1=0.0, scalar2=None, op0=Alu.is_equal)
            gtm = wk.tile([128, 128], F32)
            nc.vector.tensor_tensor(out=gtm[:], in0=t1m[:], in1=t2m[:], op=Alu.mult)
            npredm = wk.tile([128, 128], F32)
            nc.vector.tensor_scalar(out=npredm[:], in0=predm[:], scalar1=-1.0, scalar2=1.0, op0=Alu.mult, op1=Alu.add)
            anc_m = wk.tile([128, 128], F32, bufs=1)
            nc.vector.tensor_tensor(out=anc_m[:], in0=predm[:], in1=gtm[:], op=Alu.mult)
            pos_m = wk.tile([128, 128], F32, bufs=1)
            nc.vector.tensor_tensor(out=pos_m[:], in0=gtm[:], in1=npredm[:], op=Alu.mult)
            neg_m = wk.tile([128, 128], F32, bufs=1)
            nc.vector.tensor_tensor(out=neg_m[:], in0=predm[:], in1=e0m[:], op=Alu.mult)
            masks = [anc_m, pos_m, neg_m]

            # ---------- local counts -> AllGather ----------
            rs3 = wk.tile([128, 4], F32, bufs=1)
            nc.vector.memset(rs3[:], 0.0)
            for xi, mk in enumerate(masks):
                nc.vector.tensor_reduce(rs3[:, xi : xi + 1], mk[:], axis=mybir.AxisListType.X, op=Alu.add)
            cnt_ps = psA.tile([1, 4], F32, tag="tiny")
            nc.tensor.matmul(cnt_ps[:], ones_c[:], rs3[:], start=True, stop=True)
            cnt_sb = wk.tile([1, 4], F32)
            nc.scalar.copy(cnt_sb[:], cnt_ps[:])
            nc.sync.dma_start(cnt_loc, cnt_sb[:])
            nc.gpsimd.collective_compute(
                "AllGather", Alu.bypass, replica_groups=groups, ins=[cnt_loc], outs=[cnt_all]
            )

            # ============ SECTION 2: overlap window (AG-independent) ============
            # Newton-refined inverse norm helper: HW ACT sqrt is ~4e-3 relative
            # error (65536 ULP table budget); one rsqrt Newton step on the
            # combined sqrt+reciprocal estimate brings row scales to ~1e-5.
            def inv_norm(ssq, eps, tagp, P=128, Fn=8):
                # ssq is an AP [P, Fn]
                sq = wk.tile([P, Fn], F32, name=f"sqr{tagp}", tag="nsq")
                nc.scalar.sqrt(sq[:], ssq)
                nc.vector.tensor_scalar(out=sq[:], in0=sq[:], scalar1=eps, scalar2=None, op0=Alu.max)
                inv0 = wk.tile([P, Fn], F32, name=f"inv0{tagp}", tag="ninv0")
                nc.vector.reciprocal(inv0[:], sq[:])
                t1 = wk.tile([P, Fn], F32, name=f"nt1{tagp}", tag="nt1")
                nc.vector.tensor_tensor(out=t1[:], in0=inv0[:], in1=inv0[:], op=Alu.mult)
                nc.vector.tensor_tensor(out=t1[:], in0=t1[:], in1=ssq, op=Alu.mult)
                nc.vector.tensor_scalar(out=t1[:], in0=t1[:], scalar1=-0.5, scalar2=1.5, op0=Alu.mult, op1=Alu.add)
                nc.vector.tensor_tensor(out=inv0[:], in0=inv0[:], in1=t1[:], op=Alu.mult)
                return inv0

            # selection crossing-search, combined over the 3 specs into one
            # 7-group index row (anc:1 group, pos:3, neg:3)
            GRPS = [(0, 0), (1, 0), (1, 1), (1, 2), (2, 0), (2, 1), (2, 2)]
            NG = 7
            IDXF = wk.tile([128, NG], F32, name="IDXF", tag="IDXF", bufs=1)
            for xi, (mk, ngrp) in enumerate(((anc_m, 1), (pos_m, 3), (neg_m, 3))):
                scn = wk.tile([128, 128], F32, name=f"scn{xi}", tag="scn")
                nc.vector.tensor_tensor_scan(scn[:], mk[:], zeros[:], 0.0, Alu.add, Alu.add)
                ro_ps = psB.tile([128, 1], F32, name=f"rops{xi}", tag="pf")
                nc.tensor.matmul(ro_ps[:], trils[:], scn[:, 127:128], start=True, stop=True)
                rowoff = wk.tile([128, 1], F32, name=f"rowoff{xi}", tag="rowoff")
                nc.scalar.copy(rowoff[:], ro_ps[:])
                Pg = wk.tile([128, 128], F32, name=f"Pg{xi}", tag="Pg")
                nc.vector.tensor_scalar(out=Pg[:], in0=scn[:], scalar1=rowoff[:], scalar2=None, op0=Alu.add)
                roT_ps = psB.tile([128, 128], F32, name=f"roT{xi}", tag="pf")
                nc.tensor.transpose(roT_ps[0:1, :], rowoff[:], ident[:])
                roT = wk.tile([1, 128], F32, name=f"roTs{xi}", tag="roT")
                nc.scalar.copy(roT[:], roT_ps[0:1, :])
                rb_ps = psB.tile([128, 128], F32, name=f"rb{xi}", tag="pf")
                nc.tensor.matmul(rb_ps[:], ones_r128[:], roT[:], start=True, stop=True)
                RB = wk.tile([128, 128], F32, name=f"RB{xi}", tag="RB")
                nc.scalar.copy(RB[:], rb_ps[:])
                gg0 = 0 if xi == 0 else (1 if xi == 1 else 4)
                for cch in range(ngrp):
                    sio = siota3[:, cch : cch + 1]
                    cmp1 = wk.tile([128, 128], F32, name=f"cmp1{xi}{cch}", tag="cmp1")
                    nc.vector.tensor_scalar(out=cmp1[:], in0=RB[:], scalar1=sio, scalar2=None, op0=Alu.is_le)
                    rc = wk.tile([128, 1], F32, name=f"rc{xi}{cch}", tag="rc")
                    nc.vector.tensor_reduce(rc[:], cmp1[:], axis=mybir.AxisListType.X, op=Alu.add)
                    rcT_ps = psB.tile([128, 128], F32, name=f"rcT{xi}{cch}", tag="pf")
                    nc.tensor.transpose(rcT_ps[0:1, :], rc[:], ident[:])
                    rcT = wk.tile([1, 128], F32, name=f"rcTs{xi}{cch}", tag="rcT")
                    nc.scalar.copy(rcT[:], rcT_ps[0:1, :])
                    rcb_ps = psB.tile([128, 128], F32, name=f"rcb{xi}{cch}", tag="pf")
                    nc.tensor.matmul(rcb_ps[:], ones_r128[:], rcT[:], start=True, stop=True)
                    Omat = wk.tile([128, 128], F32, name=f"O{xi}{cch}", tag="Omat")
                    nc.vector.tensor_scalar(out=Omat[:], in0=rcb_ps[:], scalar1=riota1[:], scalar2=None, op0=Alu.is_equal)
                    prow_ps = psB.tile([128, 128], F32, name=f"prw{xi}{cch}", tag="pf")
                    nc.tensor.matmul(prow_ps[:], Omat[:], Pg[:], start=True, stop=True)
                    cmp2 = wk.tile([128, 128], F32, name=f"cmp2{xi}{cch}", tag="cmp2")
                    nc.vector.tensor_scalar(out=cmp2[:], in0=prow_ps[:], scalar1=sio, scalar2=None, op0=Alu.is_le)
                    wc = wk.tile([128, 1], F32, name=f"wc{xi}{cch}", tag="wc")
                    nc.vector.tensor_reduce(wc[:], cmp2[:], axis=mybir.AxisListType.X, op=Alu.add)
                    idxc = wk.tile([128, 1], F32, name=f"idxc{xi}{cch}", tag="idxc")
                    nc.vector.tensor_scalar(out=idxc[:], in0=rc[:], scalar1=128.0, scalar2=-128.0, op0=Alu.mult, op1=Alu.add)
                    nc.vector.tensor_tensor(out=IDXF[:, gg0 + cch : gg0 + cch + 1], in0=idxc[:], in1=wc[:], op=Alu.add)
            ixT_ps = psB.tile([8, 128], F32, name="ixT", tag="pf")
            nc.tensor.transpose(ixT_ps[0:NG, :], IDXF[:], ident[:])
            ixT = wk.tile([8, 128], F32, name="ixTs", tag="ixT", bufs=1)
            nc.scalar.copy(ixT[0:NG, :], ixT_ps[0:NG, :])
            ixTi = wk.tile([8, 128], dt.int32, name="ixTi", tag="ixTi", bufs=1)
            nc.vector.tensor_copy(ixTi[0:NG, :], ixT[0:NG, :])
            idxrow = wk.tile([1, NG * 128], dt.int32, name="idxrow", tag="idxrow", bufs=1)
            nc.sync.dma_start(idxrow[:], ixTi[0:NG, :])
            gat = wk.tile([128, NG, 128], F32, name="gat", tag="gat", bufs=1)
            nc.vector.memset(gat[:], 0.0)
            nc.gpsimd.indirect_dma_start(
                out=gat[:],
                out_offset=None,
                in_=embp,
                in_offset=bass.IndirectOffsetOnAxis(ap=idxrow[:], axis=0),
                bounds_check=NPIX - 1,
                oob_is_err=False,
            )
            # normalize gathered rows (eps 1e-12 + Newton)
            gsq = wk.tile([128, NG * 128], F32, name="gsq", tag="gsq", bufs=1)
            gv = gat[:].rearrange("p g d -> p (g d)")
            nc.vector.tensor_tensor(out=gsq[:], in0=gv, in1=gv, op=Alu.mult)
            gss = wk.tile([128, NG], F32, name="gss", tag="gss", bufs=1)
            nc.vector.tensor_reduce(
                gss[:], gsq[:].rearrange("p (g d) -> p g d", g=NG),
                axis=mybir.AxisListType.X, op=Alu.add,
            )
            ginv = inv_norm(gss[:], 1e-12, "g", Fn=NG)
            for g in range(NG):
                nc.vector.tensor_scalar(
                    out=gat[:, g, :], in0=gat[:, g, :],
                    scalar1=ginv[:, g : g + 1], scalar2=None, op0=Alu.mult,
                )

            # ============ SECTION 3: post-AllGather ============
            ca = wk.tile([8, 4], F32, bufs=1)
            nc.sync.dma_start(ca[:], cnt_all)
            g0_ps = psA.tile([1, 4], F32, tag="tiny")
            nc.tensor.matmul(g0_ps[:], prefmask[:], ca[:], start=True, stop=True)
            g0r = wk.tile([1, 4], F32, bufs=1)
            nc.scalar.copy(g0r[:], g0_ps[:])
            tot_ps = psA.tile([1, 4], F32, tag="tiny")
            nc.tensor.matmul(tot_ps[:], ones_c[0:8, :], ca[:], start=True, stop=True)
            totr = wk.tile([1, 4], F32, bufs=1)
            nc.scalar.copy(totr[:], tot_ps[:])
            comb = wk.tile([1, 12], F32, bufs=1)
            # comb = [g0c(4) | srow(4) | cntf(4)]
            nc.vector.tensor_tensor(out=comb[:, 0:4], in0=g0r[:], in1=kvec[:], op=Alu.min)
            nc.vector.tensor_tensor(out=comb[:, 4:8], in0=kvec[:], in1=g0r[:], op=Alu.subtract)
            nc.vector.tensor_scalar(out=comb[:, 4:8], in0=comb[:, 4:8], scalar1=0.0, scalar2=384.0, op0=Alu.max, op1=Alu.min)
            nc.vector.tensor_tensor(out=comb[:, 8:12], in0=totr[:], in1=kvec[:], op=Alu.min)
            bc_ps = psA.tile([128, 12], F32, tag="tiny")
            nc.tensor.matmul(bc_ps[:], ones_r128[:], comb[:], start=True, stop=True)
            bcast = wk.tile([128, 12], F32, bufs=1)
            nc.scalar.copy(bcast[:], bc_ps[:])
            # bcast cols: 0-3 g0c, 4-7 srow, 8-11 cntf (broadcast per partition)

            # scatter indices + one indirect scatter into the dense table
            IDX2 = wk.tile([128, NG], F32, name="IDX2", tag="IDX2", bufs=1)
            for gg, (xi, g) in enumerate(GRPS):
                base = (ANC0, POS0, NEG0)[xi]
                sio = siota3[:, g : g + 1]
                vkp = wk.tile([128, 1], F32, name=f"vkp{gg}", tag="vkp")
                nc.vector.tensor_scalar(out=vkp[:], in0=sio, scalar1=bcast[:, 4 + xi : 5 + xi], scalar2=None, op0=Alu.is_lt)
                nc.vector.tensor_scalar(out=vkp[:], in0=vkp[:], scalar1=-100000.0, scalar2=100000.0, op0=Alu.mult, op1=Alu.add)
                idc = wk.tile([128, 1], F32, name=f"idc{gg}", tag="idc")
                nc.vector.tensor_scalar(out=idc[:], in0=sio, scalar1=bcast[:, xi : xi + 1], scalar2=float(base), op0=Alu.add, op1=Alu.add)
                nc.vector.tensor_tensor(out=IDX2[:, gg : gg + 1], in0=idc[:], in1=vkp[:], op=Alu.add)
            ix2_ps = psB.tile([8, 128], F32, name="ix2", tag="pf")
            nc.tensor.transpose(ix2_ps[0:NG, :], IDX2[:], ident[:])
            ix2 = wk.tile([8, 128], F32, name="ix2s", tag="ix2s", bufs=1)
            nc.scalar.copy(ix2[0:NG, :], ix2_ps[0:NG, :])
            ix2i = wk.tile([8, 128], dt.int32, name="ix2i", tag="ix2i", bufs=1)
            nc.vector.tensor_copy(ix2i[0:NG, :], ix2[0:NG, :])
            idx2row = wk.tile([1, NG * 128], dt.int32, name="idx2row", tag="idx2row", bufs=1)
            nc.sync.dma_start(idx2row[:], ix2i[0:NG, :])
            nc.gpsimd.indirect_dma_start(
                out=contrib,
                out_offset=bass.IndirectOffsetOnAxis(ap=idx2row[:], axis=0),
                in_=gat[:],
                in_offset=None,
                bounds_check=CTOT - 1,
                oob_is_err=False,
            )

            nc.gpsimd.collective_compute(
                "AllReduce", Alu.add, replica_groups=groups, ins=[contrib], outs=[contrib_o]
            )

            # ---- work overlapping the AllReduce ----
            # memory tables: normalize (eps 1e-8 + Newton) and transpose to
            # bf16 U tables [128, 1024] (cols 1000.. are padding, never read)
            tabs = []
            for which, mem in ((0, pmall), (1, nmall)):
                UT = big.tile([128, 1024], BF16, name=f"UT{which}", tag=f"UT{which}")
                mflat = mem[:].rearrange("p i d -> p (i d)")
                msq = wk.tile([128, 1024], F32, name=f"msq{which}", tag="msq")
                nc.vector.tensor_tensor(out=msq[:], in0=mflat, in1=mflat, op=Alu.mult)
                mss = wk.tile([128, 8], F32, name=f"mss{which}", tag="mss")
                nc.vector.tensor_reduce(
                    mss[:], msq[:].rearrange("p (i d) -> p i d", i=8),
                    axis=mybir.AxisListType.X, op=Alu.add,
                )
                minv = inv_norm(mss[:], 1e-8, f"m{which}")
                for i in range(8):
                    nc.vector.tensor_scalar(
                        out=mem[:, i, :], in0=mem[:, i, :],
                        scalar1=minv[:, i : i + 1], scalar2=None, op0=Alu.mult,
                    )
                    tp = psB.tile([128, 128], BF16, name=f"tp{which}{i}", tag="ptrb")
                    nc.tensor.transpose(tp[:], mem[:, i, :], identb[:])
                    nc.scalar.copy(UT[:, 128 * i : 128 * (i + 1)], tp[:])
                tabs.append(UT)
            U_posT, U_negT = tabs

            # column-validity masks for the merge (free-dim iota vs counts)
            maskp = wk.tile([128, 384], dt.uint8, name="maskp", bufs=1)
            nc.vector.tensor_scalar(out=maskp[:, 0:KP], in0=iotaf[:, 0:KP], scalar1=bcast[:, 9:10], scalar2=None, op0=Alu.is_lt)
            maskn = wk.tile([128, 384], dt.uint8, name="maskn", bufs=1)
            nc.vector.tensor_scalar(out=maskn[:, 0:KP], in0=iotaf[:, 0:KP], scalar1=bcast[:, 10:11], scalar2=None, op0=Alu.is_lt)
            validA = wk.tile([128, 1], F32, bufs=1)
            nc.vector.tensor_scalar(out=validA[0:100, :], in0=rowiota[0:100, :], scalar1=bcast[0:100, 8:9], scalar2=None, op0=Alu.is_lt)

            # ============ SECTION 4: post-AllReduce ============
            # anchors: rows are already unit (or zero) — no renormalize.
            canc = wk.tile([128, 128], F32, bufs=1)
            nc.sync.dma_start(canc[0:100, :], contrib_o[0:100, :])
            ancT_ps = psB.tile([128, 100], F32, tag="ptr")
            nc.tensor.transpose(ancT_ps[:], canc[0:100, :], ident[0:100, 0:100])
            ancT = wk.tile([128, 100], BF16, bufs=1)
            nc.scalar.copy(ancT[:], ancT_ps[:])

            # merge gathered pos/neg rows into the U tables in T-space
            for which, UT, msk, cbase in ((0, U_posT, maskp, POS0), (1, U_negT, maskn, NEG0)):
                for ci, (r0, rn) in enumerate(((0, 128), (128, 128), (256, KP - 256))):
                    src = wk.tile([128, 128], F32, name=f"mg{which}{ci}", tag="mg")
                    nc.sync.dma_start(src[0:rn, :], contrib_o[cbase + r0 : cbase + r0 + rn, :])
                    tp = psB.tile([128, 128], F32, name=f"mgt{which}{ci}", tag="ptr")
                    nc.tensor.transpose(tp[:, 0:rn], src[0:rn, :], ident[0:rn, 0:rn])
                    stg = wk.tile([128, 128], BF16, name=f"stg{which}{ci}", tag="stg")
                    nc.scalar.copy(stg[:, 0:rn], tp[:, 0:rn])
                    nc.vector.copy_predicated(
                        out=UT[:, r0 : r0 + rn], mask=msk[:, r0 : r0 + rn], data=stg[:, 0:rn]
                    )

            # sims (bf16 matmuls, f32 PSUM)
            possim = psC.tile([KA, M], F32, name="possim", tag="sim")
            nc.tensor.matmul(possim[:, 0:512], ancT[:], U_posT[:, 0:512], start=True, stop=True)
            nc.tensor.matmul(possim[:, 512:1000], ancT[:], U_posT[:, 512:1000], start=True, stop=True)
            possim_sb = big.tile([KA, M], F32)
            nc.scalar.copy(possim_sb[:], possim[:])
            nc.sync.dma_start(possim_d, possim_sb[:])
            poffreg = nc.values_load(poff_sb[0:1, 0:1].to_broadcast((1, 1)))
            mypos = wk.tile([KA, PCOLS], F32, bufs=1)
            nc.sync.dma_start(mypos[:], possim_d[:, bass.ds(poffreg, PCOLS)])

            negsim = psC.tile([KA, M], F32, name="negsim", tag="sim")
            nc.tensor.matmul(negsim[:, 0:512], ancT[:], U_negT[:, 0:512], start=True, stop=True)
            nc.tensor.matmul(negsim[:, 512:1000], ancT[:], U_negT[:, 512:1000], start=True, stop=True)
            nbufneg = big.tile([KA, M], BF16)
            nc.scalar.mul(nbufneg[:], negsim[:], -1.0)

            amod = wk.tile([KA, PCOLS], F32, bufs=1)
            nc.vector.tensor_scalar(out=amod[:], in0=mypos[:], scalar1=MARGIN + 4.0, scalar2=None, op0=Alu.add)
            nc.vector.tensor_scalar(out=amod[:], in0=amod[:], scalar1=validA[0:100, :], scalar2=4.0, op0=Alu.mult, op1=Alu.subtract)

            # ---------- pairwise relu-sum, 3-way engine split ----------
            # (a) N_A cols: DVE plain relu (4x perf mode) + PE ones-matmul
            #     accumulate into an [1,1000] PSUM row
            # (c) N_C cols: DVE plain relu + DVE bf16 tensor_tensor add
            # (b) rest:     ScalarE activation with internal accumulator
            onesb = cst.tile([128, 1], BF16)
            nc.vector.memset(onesb[:], 1.0)
            accrow = psC.tile([1, M], F32, name="accrow", tag="accrow")
            accT = big.tile([KA, M], BF16)
            nc.vector.memset(accT[:], 0.0)
            accA = wk.tile([KA, 128], F32, bufs=1)
            scrA = big.tile([KA, M], F16)
            scrD = [big.tile([KA, M], BF16, name=f"scrD{k}") for k in range(2)]
            for i in range(N_A):
                s = scrD[i % 2]
                nc.vector.tensor_scalar(
                    out=s[:], in0=nbufneg[:],
                    scalar1=amod[:, i : i + 1], scalar2=0.0,
                    op0=Alu.add, op1=Alu.max,
                )
                nc.tensor.matmul(accrow[0:1, 0:512], onesb[0:100, :], s[:, 0:512],
                                 start=(i == 0), stop=(i == N_A - 1))
                nc.tensor.matmul(accrow[0:1, 512:1000], onesb[0:100, :], s[:, 512:1000],
                                 start=(i == 0), stop=(i == N_A - 1))
            for i in range(N_A, N_A + N_C):
                s = scrD[i % 2]
                nc.vector.tensor_scalar(
                    out=s[:], in0=nbufneg[:],
                    scalar1=amod[:, i : i + 1], scalar2=0.0,
                    op0=Alu.add, op1=Alu.max,
                )
                nc.vector.tensor_tensor(out=accT[:], in0=accT[:], in1=s[:], op=Alu.add)
            for i in range(N_A + N_C, PCOLS):
                nc.scalar.activation(
                    scrA[:], negsim[:], Act.Relu, bias=amod[:, i : i + 1], scale=-1.0,
                    accum_out=accA[:, i - N_A - N_C : i - N_A - N_C + 1],
                )

            # final reductions: accrow + accT + accA
            accrow_sb = wk.tile([1, M], F32, bufs=1)
            nc.scalar.copy(accrow_sb[:], accrow[:])
            r1 = wk.tile([KA, 2], F32, bufs=1)
            nc.vector.tensor_reduce(r1[:, 0:1], accT[:], axis=mybir.AxisListType.X, op=Alu.add)
            nc.vector.tensor_reduce(r1[:, 1:2], accA[:, 0 : PCOLS - N_A - N_C], axis=mybir.AxisListType.X, op=Alu.add)
            rsum = wk.tile([KA, 1], F32, bufs=1)
            nc.vector.tensor_tensor(out=rsum[:], in0=r1[:, 0:1], in1=r1[:, 1:2], op=Alu.add)
            tot2 = psA.tile([1, 1], F32, tag="tiny")
            nc.tensor.matmul(tot2[:], rsum[:], ones_c[0:100, :], start=True, stop=True)
            tots = wk.tile([1, 1], F32, bufs=1)
            nc.scalar.copy(tots[:], tot2[:])
            tot3 = wk.tile([1, 1], F32, bufs=1)
            nc.vector.tensor_reduce(tot3[:], accrow_sb[:], axis=mybir.AxisListType.X, op=Alu.add)
            nc.vector.tensor_tensor(out=tots[:], in0=tots[:], in1=tot3[:], op=Alu.add)
            den = wk.tile([1, 1], F32, bufs=1)
            nc.vector.tensor_scalar(out=den[:], in0=bcast[0:1, 8:9], scalar1=1.0, scalar2=1e6, op0=Alu.max, op1=Alu.mult)
            nc.vector.reciprocal(den[:], den[:])
            nc.vector.tensor_tensor(out=den[:], in0=den[:], in1=tots[:], op=Alu.mult)
            nc.sync.dma_start(out_d, den[:])

    return nc


def _host_shards(preds, embeddings, fsss_gts, pos_memory, neg_memory):
    """Build the 8 per-core input maps."""
    trils = np.tril(np.ones((128, 128), np.float32), -1).T  # lhsT[k,m]=1 iff k<m
    ident = np.eye(128, dtype=np.float32)
    identb = np.eye(128, dtype=ml_dtypes.bfloat16)
    rowiota = np.arange(128, dtype=np.float32).reshape(128, 1)
    riota1 = rowiota + 1.0
    siota3 = np.stack([np.arange(128, dtype=np.float32) + 128 * c for c in range(3)], axis=1)
    iotaf = np.broadcast_to(np.arange(384, dtype=np.float32), (128, 384)).copy()
    kvec = np.array([[KA, KP, KP, 0]], np.float32)
    pmem_pad = np.zeros((1024, D), ml_dtypes.bfloat16)
    pmem_pad[:M] = pos_memory.astype(ml_dtypes.bfloat16)
    nmem_pad = np.zeros((1024, D), ml_dtypes.bfloat16)
    nmem_pad[:M] = neg_memory.astype(ml_dtypes.bfloat16)

    in_maps = []
    for c in range(NCORES):
        psub = preds[c, :, ::4, ::4]  # [21,128,128]
        preds_t = np.ascontiguousarray(
            psub.transpose(1, 0, 2).reshape(128, C * 128)
        )
        gts_t = np.ascontiguousarray(fsss_gts[c, ::4, ::4]).astype(np.int32)
        embp = np.ascontiguousarray(
            embeddings[c].transpose(1, 2, 0).reshape(NPIX, D)
        )
        prefmask = np.zeros((8, 1), np.float32)
        prefmask[:c] = 1.0
        in_maps.append(
            {
                "preds_t": preds_t.astype(np.float32),
                "gts_t": gts_t,
                "embp": embp.astype(np.float32),
                "posmem": pmem_pad,
                "negmem": nmem_pad,
                "trils": trils.astype(np.float32),
                "ident": ident,
                "identb": identb,
                "rowiota": rowiota,
                "riota1": riota1.astype(np.float32),
                "siota3": np.ascontiguousarray(siota3),
                "iotaf": iotaf,
                "prefmask": prefmask,
                "kvec": kvec,
                "poff": np.array([[PCOLS * c]], np.int32),
            }
        )
    return in_maps


def kernel(preds, embeddings, fsss_gts, pos_memory, neg_memory):
    global LAST_EXEC_NS
    _install_patches()
    from concourse.bass_utils import run_bass_kernel_spmd

    if "nc" not in _cache:
        _cache["nc"] = _build_module()
    nc = _cache["nc"]

    in_maps = _host_shards(
        np.asarray(preds), np.asarray(embeddings), np.asarray(fsss_gts),
        np.asarray(pos_memory), np.asarray(neg_memory),
    )
    res = run_bass_kernel_spmd(nc, in_maps, list(range(NCORES)), trace=TRACE)
    LAST_EXEC_NS = res.exec_time_ns
    total = np.float32(0.0)
    for r in res.results:
        total = total + r["out"][0, 0]
    return np.float32(total)
